# revision 23
# baseline (speedup 1.0000x reference)
"""Trainium2 Bass kernel for nn_CosineLoss (data-parallel over 8 NeuronCores).

loss = -sum_n pred[n, t[n]] / ||pred[n]|| / N
       + 0.1 * mean_n (1 - ||pred[n]||)^2

(The reference adds eps=1e-9 to the norm; with randn(1000) rows norms are
~31.6, so eps is ~3e-11 relative and dropped here.)

Per core (8192 rows x 1000 cols, f32).  DMA structure is kept identical to
the proven layout (one 4MB super-tile dma per s, last super-tile split in
halves, per-half ap_gather) -- the compute side is restructured from trace
evidence (the old version bunched ~16us of DVE final work after the last
byte):

  - Row sum-of-squares: ACT Square+accum for rows {0,1,4,5}, DVE bn_stats
    (2x500) for rows {2,3,6,7}; bn rows' sumsq rebuilt with a 5-op
    vectorized fixup (sum M2 + 250*sum mean^2) using strided stat views.
  - Finals run in 4 phases spread through the stream (supertiles 0-3 at
    s=4, 4-5 at s=5, 6 at s=6, 7 in the tail):
      Sqrt(sumsq) with accum_out -> per-phase Sum(norm)   (1 ACT op)
      inv = reciprocal(norms); Sum(gath*inv), Sum(sumsq)  (host combines
      NL = N - 2*Sum(norm) + Sum(sumsq) = Sum((1-norm)^2)).
  - Output [128, 12] per core (4 phases x {gsum, rootsum, sqsum}); host
    reduces in f64.
"""

import sys

for _p in ("/root/.axon_site/_ro/trn_rl_repo", "/opt/trn_rl_repo"):
    if _p not in sys.path:
        sys.path.append(_p)

import numpy as np

N = 65536
C = 1000
NCORES = 8
R = N // NCORES          # rows per core
P = 128                  # partitions
NT = R // P              # 64 row-blocks per core
SUP = 8                  # rows per partition per super-tile
NSUP = NT // SUP         # 8 super-tiles per core
NORM_FACTOR = 0.1
NPH = 4
# final phases: (first supertile, last supertile, phase idx)
PHASES = [(0, 4, 0), (4, 6, 1), (6, 7, 2), (7, 8, 3)]
NBN = 4                  # bn rows per super-tile ({2,3,6,7})

_STATE = {}


def _build_program():
    import concourse.bacc as bacc
    import concourse.bass as bass
    import concourse.mybir as mybir
    import concourse.tile as tile

    f32 = mybir.dt.float32
    i16 = mybir.dt.int16
    AF = mybir.ActivationFunctionType
    ALU = mybir.AluOpType
    AX = mybir.AxisListType

    nc = bacc.Bacc(
        "TRN2",
        target_bir_lowering=False,
        debug=False,
        enable_asserts=False,
        num_devices=NCORES,
    )

    pred_d = nc.dram_tensor("pred", [R, C], f32, kind="ExternalInput").ap()
    tgt_d = nc.dram_tensor("tgt", [P, NT], i16, kind="ExternalInput").ap()
    m128_d = nc.dram_tensor("m128", [P, 4 * 16], f32, kind="ExternalInput").ap()
    out_d = nc.dram_tensor("out", [P, 3 * NPH], f32, kind="ExternalOutput").ap()

    # [R, C] viewed as [p, supertile, r*c]: row = s*1024 + p*8 + r, so each
    # partition's DMA run per super-tile is 32KB contiguous.
    pred_v = pred_d.rearrange("(s p r) c -> p s (r c)", p=P, r=SUP)

    with tile.TileContext(nc) as tc:
        from contextlib import ExitStack

        with ExitStack() as ctx:
            data_pool = ctx.enter_context(tc.tile_pool(name="data", bufs=5))
            work_pool = ctx.enter_context(tc.tile_pool(name="work", bufs=2))
            persist = ctx.enter_context(tc.tile_pool(name="persist", bufs=1))

            # s=0 pred DMA first on the queue; tgt/m128 land behind it
            # (~20us) which is still before the first gather can run.
            data0 = data_pool.tile([P, SUP * C], f32, tag="data")
            nc.sync.dma_start(data0[:], pred_v[:, 0, :])
            tgt_t = persist.tile([P, NT], i16)
            nc.sync.dma_start(tgt_t[:], tgt_d[:])
            m128_t = persist.tile([P, 4 * 16], f32)
            nc.sync.dma_start(m128_t[:], m128_d[:])

            # Preload the sqrt_and_others ACT table set while ACT is idle;
            # Square is a filler in every set, so no mid-kernel set switch.
            dummy = persist.tile([P, 1], f32)
            nc.gpsimd.memset(dummy[:], 1.0)
            dummy2 = persist.tile([P, 1], f32)
            nc.scalar.activation(dummy2[:], dummy[:], AF.Sqrt)

            sumsq = persist.tile([P, NT], f32)
            gath = persist.tile([P, NT], f32)
            stats_t = persist.tile([P, NSUP * NBN * 12], f32)
            norms_j = persist.tile([P, NT], f32)   # sqrt outputs (junk)
            inv_t = persist.tile([P, NT], f32)
            gj_t = persist.tile([P, NT], f32)
            fm_t = persist.tile([P, 16], f32)
            f2_t = persist.tile([P, 16], f32)
            ft_t = persist.tile([P, 16], f32)
            acc67 = persist.tile([P, 2], f32)
            out_t = persist.tile([P, 3 * NPH], f32)

            def emit_fixup(jb0, jb1, ss_view):
                """sumsq for bn stat blocks jb0..jb1:
                sum M2 + 250 * sum mean^2, from bn_stats output layout
                (per bn block: 4 chunks x (count, mean, M2))."""
                nb = jb1 - jb0
                mv = stats_t[:, 12 * jb0 : 12 * jb1].rearrange(
                    "p (b c k) -> p b c k", c=4, k=3
                )
                fx = work_pool.tile([P, 4 * 16], f32, tag="fx")
                nc.vector.tensor_mul(fx[:, : 4 * nb], mv[:, :, :, 1:2], mv[:, :, :, 1:2])
                nc.vector.tensor_reduce(
                    fm_t[:, :nb],
                    fx[:, : 4 * nb].rearrange("p (b c) -> p b c", c=4),
                    AX.X,
                    ALU.add,
                )
                nc.vector.tensor_reduce(
                    f2_t[:, :nb], mv[:, :, :, 2:3], AX.XY, ALU.add
                )
                nc.vector.tensor_scalar_mul(ft_t[:, :nb], fm_t[:, :nb], 250.0)
                nc.vector.tensor_add(ss_view, ft_t[:, :nb], f2_t[:, :nb])

            def bn_view(s0, s1):
                return sumsq[:].rearrange("p (x b) -> p x b", b=4)[
                    :, 2 * s0 : 2 * s1, 2:4
                ]

            def emit_final(s0, s1, ph):
                c0, c1 = SUP * s0, SUP * s1
                nc.scalar.activation(
                    norms_j[:, c0:c1], sumsq[:, c0:c1], AF.Sqrt,
                    accum_out=out_t[:, NPH + ph : NPH + ph + 1],
                )
                nc.vector.reciprocal(inv_t[:, c0:c1], norms_j[:, c0:c1])
                nc.vector.tensor_mul(gj_t[:, c0:c1], gath[:, c0:c1], inv_t[:, c0:c1])
                nc.vector.tensor_reduce(
                    out_t[:, ph : ph + 1], gj_t[:, c0:c1], AX.X, ALU.add
                )
                nc.vector.tensor_reduce(
                    out_t[:, 2 * NPH + ph : 2 * NPH + ph + 1],
                    sumsq[:, c0:c1],
                    AX.X,
                    ALU.add,
                )

            def emit_gather(data, e0, ne, tcol0, nblk, gcol0):
                g16 = work_pool.tile([P, 4 * 16], f32, tag="g16")
                nc.gpsimd.ap_gather(
                    g16[:, : nblk * 16],
                    data[:, e0 : e0 + ne],
                    tgt_t[:, tcol0 : tcol0 + nblk],
                    channels=P,
                    num_elems=ne,
                    d=1,
                    num_idxs=nblk * 16,
                )
                gm = work_pool.tile([P, 4 * 16], f32, tag="gm")
                nc.vector.tensor_mul(
                    gm[:, : nblk * 16], g16[:, : nblk * 16], m128_t[:, : nblk * 16]
                )
                nc.vector.tensor_reduce(
                    gath[:, gcol0 : gcol0 + nblk],
                    gm[:, : nblk * 16].rearrange("p (b i) -> p b i", i=16),
                    AX.X,
                    ALU.add,
                )

            def emit_square(data, b, j, e0=None, ne=C, acc=None):
                scrA = work_pool.tile([P, C], f32, tag="scrA")
                if e0 is None:
                    e0 = b * C
                nc.scalar.activation(
                    scrA[:, :ne],
                    data[:, e0 : e0 + ne],
                    AF.Square,
                    accum_out=acc if acc is not None else sumsq[:, j : j + 1],
                )

            def emit_bn(data, jb, e0):
                nc.vector.bn_stats(
                    stats_t[:, 12 * jb : 12 * jb + 6], data[:, e0 : e0 + 500]
                )
                nc.vector.bn_stats(
                    stats_t[:, 12 * jb + 6 : 12 * jb + 12],
                    data[:, e0 + 500 : e0 + 1000],
                )

            for s in range(NSUP):
                if s == 0:
                    data = data0  # DMA already issued first
                elif s == NSUP - 1:
                    data = data_pool.tile([P, SUP * C], f32, tag="data")
                    # Fine-grained chunks so only row 7's work remains after
                    # the last byte: [rows 0-3, 4, 5, then half-rows of 6
                    # and 7 so their ACT/DVE halves start as data arrives].
                    for e0, ne in ((0, 4 * C), (4 * C, C), (5 * C, C),
                                   (6 * C, 500), (6 * C + 500, 500),
                                   (7 * C, 500), (7 * C + 500, 500)):
                        nc.sync.dma_start(
                            data[:, e0 : e0 + ne], pred_v[:, s, e0 : e0 + ne]
                        )
                else:
                    data = data_pool.tile([P, SUP * C], f32, tag="data")
                    nc.sync.dma_start(data[:], pred_v[:, s, :])

                if s < NSUP - 1:
                    for h in range(2):
                        emit_gather(data, 4 * C * h, 4 * C, SUP * s + 4 * h, 4,
                                    SUP * s + 4 * h)
                    for b in range(SUP):
                        j = SUP * s + b
                        if b % 4 < 2:
                            emit_square(data, b, j)
                        else:
                            jb = NBN * s + 2 * (b // 4) + (b % 4) - 2
                            emit_bn(data, jb, b * C)
                    # Spread fixup+finals through the stream instead of
                    # bunching them after the last byte.
                    if s == 4:
                        emit_fixup(0, 16, bn_view(0, 4))
                        emit_final(0, 4, 0)
                    if s == 5:
                        emit_fixup(16, 24, bn_view(4, 6))
                        emit_final(4, 6, 1)
                    if s == 6:
                        emit_fixup(24, 28, bn_view(6, 7))
                        emit_final(6, 7, 2)
                else:
                    # Last super-tile, in data-arrival order.  Rows 2,3,6
                    # are bn (their fixups run before the last byte lands);
                    # row 7 is two ACT half-squares, so the post-last-byte
                    # chain is just ACT(500)x2 -> add -> final chain.
                    j0 = SUP * s
                    emit_gather(data, 0, 4 * C, j0, 4, j0)
                    emit_square(data, 0, j0 + 0)
                    emit_square(data, 1, j0 + 1)
                    emit_bn(data, 28, 2 * C)
                    emit_bn(data, 29, 3 * C)
                    emit_fixup(28, 30, sumsq[:, j0 + 2 : j0 + 4])
                    emit_gather(data, 4 * C, C, j0 + 4, 1, j0 + 4)
                    emit_square(data, 4, j0 + 4)
                    emit_gather(data, 5 * C, C, j0 + 5, 1, j0 + 5)
                    emit_square(data, 5, j0 + 5)
                    emit_gather(data, 6 * C, C, j0 + 6, 1, j0 + 6)
                    emit_bn(data, 30, 6 * C)
                    emit_fixup(30, 31, sumsq[:, j0 + 6 : j0 + 7])
                    emit_gather(data, 7 * C, C, j0 + 7, 1, j0 + 7)
                    emit_square(data, 7, j0 + 7, e0=7 * C, ne=500,
                                acc=acc67[:, 0:1])
                    emit_square(data, 7, j0 + 7, e0=7 * C + 500, ne=500,
                                acc=acc67[:, 1:2])
                    nc.vector.tensor_add(
                        sumsq[:, j0 + 7 : j0 + 8], acc67[:, 0:1], acc67[:, 1:2]
                    )

            # Tail phase: super-tile 7 finals only.
            emit_final(7, 8, 3)
            nc.sync.dma_start(out_d[:], out_t[:])

    nc.compile()
    return nc


def _host_shard(prediction, target):
    """Build per-core input maps."""
    prediction = np.asarray(prediction, dtype=np.float32)
    target = np.asarray(target)

    m128 = (
        (np.arange(4 * 16)[None, :] % 16) == (np.arange(P)[:, None] % 16)
    ).astype(np.float32)

    in_maps = []
    for k in range(NCORES):
        pred_k = np.ascontiguousarray(prediction[k * R : (k + 1) * R])
        t_k = target[k * R : (k + 1) * R].astype(np.int64)
        # Device row layout: row = s*1024 + p*8 + r.  Column j = 8*s + r of
        # tgt/sumsq.  Gather offset within a 4-block half is (r % 4) * C.
        tk = t_k.reshape(NSUP, P, SUP)  # [s, p, r]
        tk = np.transpose(tk, (1, 0, 2)).reshape(P, NT)  # [p, 8s+r]
        off = (np.arange(NT) % 4) * C  # [64]
        off = off.copy()
        # super-tile 7 gathers in chunks [rows 0-3, 4, 5, 6, 7]
        off[SUP * (NSUP - 1) :] = [0, C, 2 * C, 3 * C, 0, 0, 0, 0]
        tgt_k = (tk + off[None, :]).astype(np.int16)
        in_maps.append({"pred": pred_k, "tgt": tgt_k, "m128": m128})
    return in_maps


def _combine(results):
    """results: list of {'out': [128, 12]} per core -> scalar f32 loss."""
    outs = np.stack([np.asarray(r["out"], dtype=np.float64) for r in results])
    G = outs[:, :, 0:NPH].sum()
    RS = outs[:, :, NPH : 2 * NPH].sum()
    SQ = outs[:, :, 2 * NPH : 3 * NPH].sum()
    NL = N - 2.0 * RS + SQ  # sum over n of (1 - norm_n)^2
    loss = -G / N + NORM_FACTOR * (NL / N)
    return np.float32(loss)


def get_nc():
    if "nc" not in _STATE:
        _STATE["nc"] = _build_program()
    return _STATE["nc"]


def _get_runner():
    """Cached jitted shard_map runner (mirrors bass2jax.run_bass_via_pjrt,
    but reusable across kernel() calls without re-lowering)."""
    if "runner" in _STATE:
        return _STATE["runner"]

    import jax
    from jax.experimental.shard_map import shard_map
    from jax.sharding import Mesh, PartitionSpec

    import concourse.mybir as mybir
    from concourse import bass2jax

    nc = get_nc()
    bass2jax.install_neuronx_cc_hook()

    partition_name = nc.partition_id_tensor.name if nc.partition_id_tensor else None
    in_names, out_names, out_avals, zero_outs = [], [], [], []
    for alloc in nc.m.functions[0].allocations:
        if not isinstance(alloc, mybir.MemoryLocationSet):
            continue
        name = alloc.memorylocations[0].name
        if alloc.kind == "ExternalInput":
            if name != partition_name:
                in_names.append(name)
        elif alloc.kind == "ExternalOutput":
            out_names.append(name)
            shape = tuple(alloc.tensor_shape)
            dtype = mybir.dt.np(alloc.dtype)
            out_avals.append(jax.core.ShapedArray(shape, dtype))
            zero_outs.append(np.zeros(shape, dtype))
    n_params = len(in_names)
    n_outs = len(out_avals)
    all_in = in_names + out_names + ([partition_name] if partition_name else [])

    def _body(*args):
        operands = list(args)
        if partition_name is not None:
            operands.append(bass2jax.partition_id_tensor())
        outs = bass2jax._bass_exec_p.bind(
            *operands,
            out_avals=tuple(out_avals),
            in_names=tuple(all_in),
            out_names=tuple(out_names),
            lowering_input_output_aliases=(),
            sim_require_finite=True,
            sim_require_nnan=True,
            nc=nc,
        )
        return tuple(outs)

    devices = jax.devices()[:NCORES]
    mesh = Mesh(np.asarray(devices), ("core",))
    sharded = jax.jit(
        shard_map(
            _body,
            mesh=mesh,
            in_specs=(PartitionSpec("core"),) * (n_params + n_outs),
            out_specs=(PartitionSpec("core"),) * len(out_names),
            check_rep=False,
        ),
        donate_argnums=tuple(range(n_params, n_params + n_outs)),
        keep_unused=True,
    )

    def run(in_maps):
        concat_in = [
            np.concatenate([np.asarray(in_maps[c][n]) for c in range(NCORES)], axis=0)
            for n in in_names
        ]
        concat_zeros = [
            np.zeros((NCORES * z.shape[0], *z.shape[1:]), z.dtype) for z in zero_outs
        ]
        out_arrs = sharded(*concat_in, *concat_zeros)
        return [
            {
                name: np.asarray(out_arrs[i]).reshape(NCORES, *out_avals[i].shape)[c]
                for i, name in enumerate(out_names)
            }
            for c in range(NCORES)
        ]

    _STATE["runner"] = run
    return run


def kernel(prediction, target):
    in_maps = _host_shard(prediction, target)
    results = _get_runner()(in_maps)
    return _combine(results)


# revision 25
# speedup vs baseline: 1.0076x; 1.0076x over previous
"""Trainium2 Bass kernel for nn_CosineLoss (data-parallel over 8 NeuronCores).

loss = -sum_n pred[n, t[n]] / ||pred[n]|| / N
       + 0.1 * mean_n (1 - ||pred[n]||)^2

(The reference adds eps=1e-9 to the norm; with randn(1000) rows norms are
~31.6, so eps is ~3e-11 relative and dropped here.)

Per core (8192 rows x 1000 cols, f32).  DMA structure is kept identical to
the proven layout (one 4MB super-tile dma per s, last super-tile split in
halves, per-half ap_gather) -- the compute side is restructured from trace
evidence (the old version bunched ~16us of DVE final work after the last
byte):

  - Row sum-of-squares: ACT Square+accum for rows {0,1,4,5}, DVE bn_stats
    (2x500) for rows {2,3,6,7}; bn rows' sumsq rebuilt with a 5-op
    vectorized fixup (sum M2 + 250*sum mean^2) using strided stat views.
  - Finals run in 4 phases spread through the stream (supertiles 0-3 at
    s=4, 4-5 at s=5, 6 at s=6, 7 in the tail):
      Sqrt(sumsq) with accum_out -> per-phase Sum(norm)   (1 ACT op)
      inv = reciprocal(norms); Sum(gath*inv), Sum(sumsq)  (host combines
      NL = N - 2*Sum(norm) + Sum(sumsq) = Sum((1-norm)^2)).
  - Output [128, 12] per core (4 phases x {gsum, rootsum, sqsum}); host
    reduces in f64.
"""

import sys

for _p in ("/root/.axon_site/_ro/trn_rl_repo", "/opt/trn_rl_repo"):
    if _p not in sys.path:
        sys.path.append(_p)

import numpy as np

N = 65536
C = 1000
NCORES = 8
R = N // NCORES          # rows per core
P = 128                  # partitions
NT = R // P              # 64 row-blocks per core
SUP = 8                  # rows per partition per super-tile
NSUP = NT // SUP         # 8 super-tiles per core
NORM_FACTOR = 0.1
NPH = 4
# final phases: (first supertile, last supertile, phase idx)
PHASES = [(0, 4, 0), (4, 6, 1), (6, 7, 2), (7, 8, 3)]
NBN = 4                  # bn rows per super-tile ({2,3,6,7})

_STATE = {}


def _build_program():
    import concourse.bacc as bacc
    import concourse.bass as bass
    import concourse.mybir as mybir
    import concourse.tile as tile

    f32 = mybir.dt.float32
    i16 = mybir.dt.int16
    AF = mybir.ActivationFunctionType
    ALU = mybir.AluOpType
    AX = mybir.AxisListType

    nc = bacc.Bacc(
        "TRN2",
        target_bir_lowering=False,
        debug=False,
        enable_asserts=False,
        num_devices=NCORES,
    )

    pred_d = nc.dram_tensor("pred", [R, C], f32, kind="ExternalInput").ap()
    tgt_d = nc.dram_tensor("tgt", [P, NT], i16, kind="ExternalInput").ap()
    m128_d = nc.dram_tensor("m128", [P, 4 * 16], f32, kind="ExternalInput").ap()
    out_d = nc.dram_tensor("out", [P, 3 * NPH], f32, kind="ExternalOutput").ap()

    # [R, C] viewed as [p, supertile, r*c]: row = s*1024 + p*8 + r, so each
    # partition's DMA run per super-tile is 32KB contiguous.
    pred_v = pred_d.rearrange("(s p r) c -> p s (r c)", p=P, r=SUP)

    with tile.TileContext(nc) as tc:
        from contextlib import ExitStack

        with ExitStack() as ctx:
            data_pool = ctx.enter_context(tc.tile_pool(name="data", bufs=5))
            work_pool = ctx.enter_context(tc.tile_pool(name="work", bufs=2))
            persist = ctx.enter_context(tc.tile_pool(name="persist", bufs=1))

            # s=0 pred DMA first on the queue; tgt/m128 land behind it
            # (~20us) which is still before the first gather can run.
            data0 = data_pool.tile([P, SUP * C], f32, tag="data")
            nc.sync.dma_start(data0[:], pred_v[:, 0, :])
            tgt_t = persist.tile([P, NT], i16)
            nc.sync.dma_start(tgt_t[:], tgt_d[:])
            m128_t = persist.tile([P, 4 * 16], f32)
            nc.sync.dma_start(m128_t[:], m128_d[:])

            # Preload the sqrt_and_others ACT table set while ACT is idle;
            # Square is a filler in every set, so no mid-kernel set switch.
            dummy = persist.tile([P, 1], f32)
            nc.gpsimd.memset(dummy[:], 1.0)
            dummy2 = persist.tile([P, 1], f32)
            nc.scalar.activation(dummy2[:], dummy[:], AF.Sqrt)

            sumsq = persist.tile([P, NT], f32)
            gath = persist.tile([P, NT], f32)
            stats_t = persist.tile([P, NSUP * NBN * 12], f32)
            norms_j = persist.tile([P, NT], f32)   # sqrt outputs (junk)
            inv_t = persist.tile([P, NT], f32)
            gj_t = persist.tile([P, NT], f32)
            fm_t = persist.tile([P, 16], f32)
            f2_t = persist.tile([P, 16], f32)
            ft_t = persist.tile([P, 16], f32)
            acc67 = persist.tile([P, 2], f32)
            out_t = persist.tile([P, 3 * NPH], f32)

            def emit_fixup(jb0, jb1, ss_view):
                """sumsq for bn stat blocks jb0..jb1:
                sum M2 + 250 * sum mean^2, from bn_stats output layout
                (per bn block: 4 chunks x (count, mean, M2))."""
                nb = jb1 - jb0
                mv = stats_t[:, 12 * jb0 : 12 * jb1].rearrange(
                    "p (b c k) -> p b c k", c=4, k=3
                )
                fx = work_pool.tile([P, 4 * 16], f32, tag="fx")
                nc.vector.tensor_mul(fx[:, : 4 * nb], mv[:, :, :, 1:2], mv[:, :, :, 1:2])
                nc.vector.tensor_reduce(
                    fm_t[:, :nb],
                    fx[:, : 4 * nb].rearrange("p (b c) -> p b c", c=4),
                    AX.X,
                    ALU.add,
                )
                nc.vector.tensor_reduce(
                    f2_t[:, :nb], mv[:, :, :, 2:3], AX.XY, ALU.add
                )
                nc.vector.tensor_scalar_mul(ft_t[:, :nb], fm_t[:, :nb], 250.0)
                nc.vector.tensor_add(ss_view, ft_t[:, :nb], f2_t[:, :nb])

            def bn_view(s0, s1):
                return sumsq[:].rearrange("p (x b) -> p x b", b=4)[
                    :, 2 * s0 : 2 * s1, 2:4
                ]

            def emit_final(s0, s1, ph):
                c0, c1 = SUP * s0, SUP * s1
                nc.scalar.activation(
                    norms_j[:, c0:c1], sumsq[:, c0:c1], AF.Sqrt,
                    accum_out=out_t[:, NPH + ph : NPH + ph + 1],
                )
                nc.vector.reciprocal(inv_t[:, c0:c1], norms_j[:, c0:c1])
                nc.vector.tensor_mul(gj_t[:, c0:c1], gath[:, c0:c1], inv_t[:, c0:c1])
                nc.vector.tensor_reduce(
                    out_t[:, ph : ph + 1], gj_t[:, c0:c1], AX.X, ALU.add
                )
                nc.vector.tensor_reduce(
                    out_t[:, 2 * NPH + ph : 2 * NPH + ph + 1],
                    sumsq[:, c0:c1],
                    AX.X,
                    ALU.add,
                )

            def emit_gather(data, e0, ne, tcol0, nblk, gcol0):
                g16 = work_pool.tile([P, 4 * 16], f32, tag="g16")
                nc.gpsimd.ap_gather(
                    g16[:, : nblk * 16],
                    data[:, e0 : e0 + ne],
                    tgt_t[:, tcol0 : tcol0 + nblk],
                    channels=P,
                    num_elems=ne,
                    d=1,
                    num_idxs=nblk * 16,
                )
                gm = work_pool.tile([P, 4 * 16], f32, tag="gm")
                nc.vector.tensor_mul(
                    gm[:, : nblk * 16], g16[:, : nblk * 16], m128_t[:, : nblk * 16]
                )
                nc.vector.tensor_reduce(
                    gath[:, gcol0 : gcol0 + nblk],
                    gm[:, : nblk * 16].rearrange("p (b i) -> p b i", i=16),
                    AX.X,
                    ALU.add,
                )

            def emit_square(data, b, j, e0=None, ne=C, acc=None):
                scrA = work_pool.tile([P, C], f32, tag="scrA")
                if e0 is None:
                    e0 = b * C
                nc.scalar.activation(
                    scrA[:, :ne],
                    data[:, e0 : e0 + ne],
                    AF.Square,
                    accum_out=acc if acc is not None else sumsq[:, j : j + 1],
                )

            def emit_bn(data, jb, e0):
                nc.vector.bn_stats(
                    stats_t[:, 12 * jb : 12 * jb + 6], data[:, e0 : e0 + 500]
                )
                nc.vector.bn_stats(
                    stats_t[:, 12 * jb + 6 : 12 * jb + 12],
                    data[:, e0 + 500 : e0 + 1000],
                )

            for s in range(NSUP):
                if s == 0:
                    data = data0  # DMA already issued first
                elif s == NSUP - 1:
                    data = data_pool.tile([P, SUP * C], f32, tag="data")
                    # Fine-grained chunks so only row 7's work remains after
                    # the last byte: [rows 0-3, 4, 5, then half-rows of 6
                    # and 7 so their ACT/DVE halves start as data arrives].
                    for e0, ne in ((0, 4 * C), (4 * C, C), (5 * C, C),
                                   (6 * C, 500), (6 * C + 500, 500),
                                   (7 * C, 500), (7 * C + 500, 500)):
                        nc.sync.dma_start(
                            data[:, e0 : e0 + ne], pred_v[:, s, e0 : e0 + ne]
                        )
                else:
                    data = data_pool.tile([P, SUP * C], f32, tag="data")
                    nc.sync.dma_start(data[:], pred_v[:, s, :])

                if s < NSUP - 1:
                    for h in range(2):
                        emit_gather(data, 4 * C * h, 4 * C, SUP * s + 4 * h, 4,
                                    SUP * s + 4 * h)
                    for b in range(SUP):
                        j = SUP * s + b
                        if b % 4 < 2:
                            emit_square(data, b, j)
                        else:
                            jb = NBN * s + 2 * (b // 4) + (b % 4) - 2
                            emit_bn(data, jb, b * C)
                    # Spread fixup+finals through the stream instead of
                    # bunching them after the last byte.
                    if s == 4:
                        emit_fixup(0, 16, bn_view(0, 4))
                        emit_final(0, 4, 0)
                    if s == 5:
                        emit_fixup(16, 24, bn_view(4, 6))
                        emit_final(4, 6, 1)
                    if s == 6:
                        emit_fixup(24, 28, bn_view(6, 7))
                        emit_final(6, 7, 2)
                else:
                    # Last super-tile, in data-arrival order.  Rows 2,3,6
                    # are bn (their fixups run before the last byte lands);
                    # row 7 is two ACT half-squares, so the post-last-byte
                    # chain is just ACT(500)x2 -> add -> final chain.
                    j0 = SUP * s
                    emit_gather(data, 0, 4 * C, j0, 4, j0)
                    emit_square(data, 0, j0 + 0)
                    emit_square(data, 1, j0 + 1)
                    emit_bn(data, 28, 2 * C)
                    emit_bn(data, 29, 3 * C)
                    emit_fixup(28, 30, sumsq[:, j0 + 2 : j0 + 4])
                    # 2-wide gathers: each ap_gather has ~1.5-2us of fixed
                    # gpsimd cost, so five 1-wide gathers serialized past
                    # the stream end; two 2-wide ones don't.
                    emit_gather(data, 4 * C, 2 * C, j0 + 4, 2, j0 + 4)
                    emit_square(data, 4, j0 + 4)
                    emit_square(data, 5, j0 + 5)
                    emit_bn(data, 30, 6 * C)
                    emit_fixup(30, 31, sumsq[:, j0 + 6 : j0 + 7])
                    emit_gather(data, 6 * C, 2 * C, j0 + 6, 2, j0 + 6)
                    emit_square(data, 7, j0 + 7, e0=7 * C, ne=500,
                                acc=acc67[:, 0:1])
                    emit_square(data, 7, j0 + 7, e0=7 * C + 500, ne=500,
                                acc=acc67[:, 1:2])
                    nc.vector.tensor_add(
                        sumsq[:, j0 + 7 : j0 + 8], acc67[:, 0:1], acc67[:, 1:2]
                    )

            # Tail phase: super-tile 7 finals only.
            emit_final(7, 8, 3)
            nc.sync.dma_start(out_d[:], out_t[:])

    nc.compile()
    return nc


def _host_shard(prediction, target):
    """Build per-core input maps."""
    prediction = np.asarray(prediction, dtype=np.float32)
    target = np.asarray(target)

    m128 = (
        (np.arange(4 * 16)[None, :] % 16) == (np.arange(P)[:, None] % 16)
    ).astype(np.float32)

    in_maps = []
    for k in range(NCORES):
        pred_k = np.ascontiguousarray(prediction[k * R : (k + 1) * R])
        t_k = target[k * R : (k + 1) * R].astype(np.int64)
        # Device row layout: row = s*1024 + p*8 + r.  Column j = 8*s + r of
        # tgt/sumsq.  Gather offset within a 4-block half is (r % 4) * C.
        tk = t_k.reshape(NSUP, P, SUP)  # [s, p, r]
        tk = np.transpose(tk, (1, 0, 2)).reshape(P, NT)  # [p, 8s+r]
        off = (np.arange(NT) % 4) * C  # [64]
        off = off.copy()
        # super-tile 7 gathers: [rows 0-3] 4-wide, [4,5] and [6,7] 2-wide
        off[SUP * (NSUP - 1) :] = [0, C, 2 * C, 3 * C, 0, C, 0, C]
        tgt_k = (tk + off[None, :]).astype(np.int16)
        in_maps.append({"pred": pred_k, "tgt": tgt_k, "m128": m128})
    return in_maps


def _combine(results):
    """results: list of {'out': [128, 12]} per core -> scalar f32 loss."""
    outs = np.stack([np.asarray(r["out"], dtype=np.float64) for r in results])
    G = outs[:, :, 0:NPH].sum()
    RS = outs[:, :, NPH : 2 * NPH].sum()
    SQ = outs[:, :, 2 * NPH : 3 * NPH].sum()
    NL = N - 2.0 * RS + SQ  # sum over n of (1 - norm_n)^2
    loss = -G / N + NORM_FACTOR * (NL / N)
    return np.float32(loss)


def get_nc():
    if "nc" not in _STATE:
        _STATE["nc"] = _build_program()
    return _STATE["nc"]


def _get_runner():
    """Cached jitted shard_map runner (mirrors bass2jax.run_bass_via_pjrt,
    but reusable across kernel() calls without re-lowering)."""
    if "runner" in _STATE:
        return _STATE["runner"]

    import jax
    from jax.experimental.shard_map import shard_map
    from jax.sharding import Mesh, PartitionSpec

    import concourse.mybir as mybir
    from concourse import bass2jax

    nc = get_nc()
    bass2jax.install_neuronx_cc_hook()

    partition_name = nc.partition_id_tensor.name if nc.partition_id_tensor else None
    in_names, out_names, out_avals, zero_outs = [], [], [], []
    for alloc in nc.m.functions[0].allocations:
        if not isinstance(alloc, mybir.MemoryLocationSet):
            continue
        name = alloc.memorylocations[0].name
        if alloc.kind == "ExternalInput":
            if name != partition_name:
                in_names.append(name)
        elif alloc.kind == "ExternalOutput":
            out_names.append(name)
            shape = tuple(alloc.tensor_shape)
            dtype = mybir.dt.np(alloc.dtype)
            out_avals.append(jax.core.ShapedArray(shape, dtype))
            zero_outs.append(np.zeros(shape, dtype))
    n_params = len(in_names)
    n_outs = len(out_avals)
    all_in = in_names + out_names + ([partition_name] if partition_name else [])

    def _body(*args):
        operands = list(args)
        if partition_name is not None:
            operands.append(bass2jax.partition_id_tensor())
        outs = bass2jax._bass_exec_p.bind(
            *operands,
            out_avals=tuple(out_avals),
            in_names=tuple(all_in),
            out_names=tuple(out_names),
            lowering_input_output_aliases=(),
            sim_require_finite=True,
            sim_require_nnan=True,
            nc=nc,
        )
        return tuple(outs)

    devices = jax.devices()[:NCORES]
    mesh = Mesh(np.asarray(devices), ("core",))
    sharded = jax.jit(
        shard_map(
            _body,
            mesh=mesh,
            in_specs=(PartitionSpec("core"),) * (n_params + n_outs),
            out_specs=(PartitionSpec("core"),) * len(out_names),
            check_rep=False,
        ),
        donate_argnums=tuple(range(n_params, n_params + n_outs)),
        keep_unused=True,
    )

    def run(in_maps):
        concat_in = [
            np.concatenate([np.asarray(in_maps[c][n]) for c in range(NCORES)], axis=0)
            for n in in_names
        ]
        concat_zeros = [
            np.zeros((NCORES * z.shape[0], *z.shape[1:]), z.dtype) for z in zero_outs
        ]
        out_arrs = sharded(*concat_in, *concat_zeros)
        return [
            {
                name: np.asarray(out_arrs[i]).reshape(NCORES, *out_avals[i].shape)[c]
                for i, name in enumerate(out_names)
            }
            for c in range(NCORES)
        ]

    _STATE["runner"] = run
    return run


def kernel(prediction, target):
    in_maps = _host_shard(prediction, target)
    results = _get_runner()(in_maps)
    return _combine(results)
